# revision 21
# baseline (speedup 1.0000x reference)
"""Multi-head attention with KV cache on 8 Trainium2 NeuronCores.

Sharding: 8 cores = 2 batches x 4 head-groups (4 heads each, tensor-parallel
over heads / d_model slices of the projections). Out-proj partials are summed
on the host; K/V cache shards per head with no cross-device comm.

Per-core kernel (Bass/Tile):
  - projections Q^T,K^T (transposed, bf16 out) and V (natural, fp32) via f32r
    matmuls; biases fused (per-partition add for Q/K, C=1 ones-matmul for V)
  - scores computed transposed S^T[kv, q] so softmax's P^T feeds AV directly:
    2 heads row-packed per PE pass (C=64 -> tile_position (0,0)/(64,0))
  - exp on ScalarE straight from PSUM (scale=1/8 fused), causal masking via
    additive -inf bias tiles on the diagonal chunks only; fully-masked kv
    chunks are skipped entirely
  - AV: lhsT=[V|1] f32r -> attn^T plus the softmax rowsum as a free 65th row
  - normalization: recip row via exp(-ln) on ScalarE, broadcast across
    partitions on GpSimd, one DVE multiply
  - out-proj consumes attn^T directly (C=64 per head, accumulated in PSUM)
"""

import sys

for _p in ("/opt/trn_rl_repo", "/opt/trn_rl_repo/concourse"):
    if _p not in sys.path:
        sys.path.insert(0, _p)

import numpy as np
import ml_dtypes

# ---- problem constants (hardcoded per contract) ----
B = 2
TQ = 2048
DM = 1024
NH = 16
DK = 64
TP = 2048
TKV = TP + TQ          # 4096
NCORES = 8
HC = NH // (NCORES // B)   # heads per core = 4
DLOC = HC * DK             # 256 local projection dims
P = 128
QT_TILE = 512              # q tile (free dim of S^T matmuls)
NT = TQ // QT_TILE         # 4 q tiles
NKC = TKV // P             # 32 kv chunks
SCALE = 1.0 / (DK ** 0.5)  # 0.125

_BUILD_CACHE = {}
LAST_RESULT = None         # BassKernelResults of the most recent run (for test.py)


def _analyze_mask(mask):
    """Per-row prefix extents + per-(tile, chunk) classification.

    Returns (ext, plan, mb) where plan[t] = (n_chunks, {chunk: ('full'|idx)})
    and mb is the stacked [n_partial, 128, QT_TILE] f32 additive-bias array
    in S^T layout (kv_local, q_local)."""
    m = np.asarray(mask).reshape(TQ, TKV).astype(bool)
    ext = m.sum(axis=1).astype(np.int64)
    # verify prefix form: row i is ones then zeros
    idx = np.arange(TKV)[None, :]
    assert (m == (idx < ext[:, None])).all(), "mask is not prefix-form per row"

    plan = []
    biases = []
    for t in range(NT):
        qs = t * QT_TILE
        e = ext[qs:qs + QT_TILE]
        emin, emax = int(e.min()), int(e.max())
        n_chunks = (emax + P - 1) // P
        chunks = {}
        for c in range(n_chunks):
            if (c + 1) * P <= emin:
                chunks[c] = "full"
            else:
                kv_idx = c * P + np.arange(P)[:, None]        # [128, 1]
                valid = kv_idx < e[None, :]                   # [128, 512]
                bias = np.where(valid, 0.0, -3.0e38).astype(np.float32)
                chunks[c] = len(biases)
                biases.append(bias)
        plan.append((n_chunks, chunks))
    if biases:
        mb = np.stack(biases)
    else:
        mb = np.zeros((1, P, QT_TILE), np.float32)
    return ext, plan, mb


def _build(plan_key, plan, n_mb):
    import concourse.bass as bass
    import concourse.mybir as mybir
    import concourse.tile as tile
    from concourse import bacc

    F32 = mybir.dt.float32
    F32R = mybir.dt.float32r
    BF16 = mybir.dt.bfloat16
    AF = mybir.ActivationFunctionType

    nc = bacc.Bacc(trn_type="TRN2")

    # ---- DRAM I/O ----
    qT = nc.dram_tensor("qT", [DM, TQ], BF16, kind="ExternalInput")
    kT = nc.dram_tensor("kT", [DM, TQ], BF16, kind="ExternalInput")
    vT = nc.dram_tensor("vT", [DM, TQ], BF16, kind="ExternalInput")
    wqT = nc.dram_tensor("wqT", [DM, DLOC], BF16, kind="ExternalInput")
    wkT = nc.dram_tensor("wkT", [DM, DLOC], BF16, kind="ExternalInput")
    wvT = nc.dram_tensor("wvT", [DM, DLOC], BF16, kind="ExternalInput")
    woT = nc.dram_tensor("woT", [DLOC, DM], BF16, kind="ExternalInput")
    bqc = nc.dram_tensor("bqc", [P, 2], F32, kind="ExternalInput")
    bkc = nc.dram_tensor("bkc", [P, 2], F32, kind="ExternalInput")
    bvr = nc.dram_tensor("bvr", [1, DLOC], BF16, kind="ExternalInput")
    pastKT = nc.dram_tensor("pastKT", [2, P, TP], BF16, kind="ExternalInput")
    pastV = nc.dram_tensor("pastV", [HC, TP, DK], BF16, kind="ExternalInput")
    maskbias = nc.dram_tensor("maskbias", [n_mb, P, QT_TILE], F32,
                              kind="ExternalInput")
    outp = nc.dram_tensor("outp", [TQ, DM], F32, kind="ExternalOutput")
    kTnew = nc.dram_tensor("kTnew", [2, P, TQ], F32, kind="ExternalOutput")
    vnew = nc.dram_tensor("vnew", [HC, TQ, DK], BF16, kind="ExternalOutput")

    with tile.TileContext(nc) as tc:
        with (
            tc.tile_pool(name="singles", bufs=1) as singles,
            tc.tile_pool(name="stage", bufs=2) as stage,
            tc.tile_pool(name="ptpool", bufs=3) as ptpool,
            tc.tile_pool(name="mbpool", bufs=2) as mbpool,
            tc.tile_pool(name="avstage", bufs=6) as avstage,
            tc.tile_pool(name="attnT", bufs=8) as attnT_pool,
            tc.tile_pool(name="bcast", bufs=2) as bcast,
            tc.tile_pool(name="ostage", bufs=3) as ostage,
            tc.tile_pool(name="dramp", bufs=4, space="DRAM") as dramp,
            tc.tile_pool(name="gen_ps", bufs=2, space="PSUM") as gen_ps,
            tc.tile_pool(name="st_ps", bufs=2, space="PSUM") as st_ps,
            tc.tile_pool(name="av_ps", bufs=2, space="PSUM") as av_ps,
        ):
            # ---- persistent SBUF tensors ----
            wq_sb = singles.tile([P, 8, DLOC], BF16)
            wk_sb = singles.tile([P, 8, DLOC], BF16)
            wv_sb = singles.tile([P, 8, DLOC], BF16)
            wo_sb = singles.tile([DK, HC, DM], BF16)
            nc.sync.dma_start(wq_sb[:], wqT.rearrange("(c p) m -> p c m", p=P))
            nc.sync.dma_start(wk_sb[:], wkT.rearrange("(c p) m -> p c m", p=P))
            nc.sync.dma_start(wv_sb[:], wvT.rearrange("(c p) m -> p c m", p=P))
            nc.sync.dma_start(wo_sb[:], woT.rearrange("(h p) m -> p h m", p=DK))
            bq_sb = singles.tile([P, 2], F32)
            bk_sb = singles.tile([P, 2], F32)
            bv_sb = singles.tile([1, DLOC], BF16)
            nc.sync.dma_start(bq_sb[:], bqc[:])
            nc.sync.dma_start(bk_sb[:], bkc[:])
            nc.sync.dma_start(bv_sb[:], bvr[:])
            ones_sb = singles.tile([P, P], BF16)
            nc.vector.memset(ones_sb[:], 1.0)

            KT_sb = [singles.tile([P, TKV], BF16, name=f"KT{i}") for i in range(2)]
            QT_sb = [singles.tile([P, TQ], BF16, name=f"QT{i}") for i in range(2)]
            V_sb = [singles.tile([P, NKC, DK + 1], BF16, name=f"V{i}")
                    for i in range(HC)]
            kTn_sb = [singles.tile([P, TQ], F32, name=f"kTn{i}") for i in range(2)]

            for p in range(2):
                nc.sync.dma_start(KT_sb[p][:, 0:TP], pastKT[p])
            for h in range(HC):
                nc.sync.dma_start(
                    V_sb[h][:, 0:TP // P, 0:DK],
                    pastV[h].rearrange("(c p) d -> p c d", p=P),
                )
                nc.vector.memset(V_sb[h][:, :, DK:DK + 1], 1.0)

            # ---- per-512-q-tile: projections then attention (interleaved) ----
            import os as _os
            _bisect = _os.environ.get("KBISECT", "full")

            def emit_proj(ti):
                ts0 = ti * QT_TILE
                tsl = slice(ts0, ts0 + QT_TILE)

                qst = stage.tile([P, 8, QT_TILE], BF16, tag="instage",
                                 name=f"qst{ti}")
                nc.sync.dma_start(
                    qst[:], qT[:, tsl].rearrange("(c p) t -> p c t", p=P))
                for m in range(2):
                    pq = gen_ps.tile([P, 512], F32, tag="gen", name=f"pq{ti}{m}")
                    for kc in range(8):
                        nc.tensor.matmul(
                            pq[:], wq_sb[:, kc, m * P:(m + 1) * P],
                            qst[:, kc, :],
                            start=(kc == 0), stop=(kc == 7))
                    nc.vector.tensor_scalar_add(
                        QT_sb[m][:, tsl], pq[:], bq_sb[:, m:m + 1])

                kst = stage.tile([P, 8, QT_TILE], BF16, tag="instage",
                                 name=f"kst{ti}")
                nc.sync.dma_start(
                    kst[:], kT[:, tsl].rearrange("(c p) t -> p c t", p=P))
                for m in range(2):
                    pk = gen_ps.tile([P, 512], F32, tag="gen", name=f"pk{ti}{m}")
                    for kc in range(8):
                        nc.tensor.matmul(
                            pk[:], wk_sb[:, kc, m * P:(m + 1) * P],
                            kst[:, kc, :],
                            start=(kc == 0), stop=(kc == 7))
                    nc.vector.tensor_scalar_add(
                        KT_sb[m][:, TP + ts0:TP + ts0 + QT_TILE], pk[:],
                        bk_sb[:, m:m + 1])
                    nc.vector.tensor_scalar_add(
                        kTn_sb[m][:, tsl], pk[:], bk_sb[:, m:m + 1])

                vst = stage.tile([P, 8, QT_TILE], BF16, tag="instage",
                                 name=f"vst{ti}")
                nc.sync.dma_start(
                    vst[:], vT[:, tsl].rearrange("(c p) t -> p c t", p=P))
                for sub in range(4):
                    pv = gen_ps.tile([P, 512], F32, tag="gen", name=f"pv{ti}{sub}")
                    for kc in range(8):
                        nc.tensor.matmul(
                            pv[:, 0:DLOC],
                            vst[:, kc, sub * P:(sub + 1) * P],
                            wv_sb[:, kc, :],
                            start=(kc == 0), stop=False)
                    nc.tensor.matmul(
                        pv[:, 0:DLOC], ones_sb[0:1, :],
                        bv_sb[:], start=False, stop=True)
                    ci = TP // P + ti * 4 + sub
                    for h in range(HC):
                        nc.vector.tensor_copy(
                            V_sb[h][:, ci, 0:DK], pv[:, h * DK:(h + 1) * DK])

            def emit_attn(ti, do_oproj):
                qs = ti * QT_TILE
                qsl = slice(qs, qs + QT_TILE)
                n_chunks, chunk_info = plan[ti]
                att = {}
                avss = {}
                for pr in range(2):
                    ha, hb = 2 * pr, 2 * pr + 1
                    av = [av_ps.tile([P, 512], F32, tag="av", name=f"av{i}")
                          for i in range(2)]
                    pts = {}

                    def emit_st(c):
                        sT = st_ps.tile([P, 1024], F32, tag="sT", name=f"sT{c}")
                        for x in range(2):
                            nc.tensor.matmul(
                                sT[:, 512 * x:512 * x + 512],
                                KT_sb[pr][64 * x:64 * x + 64, c * P:(c + 1) * P],
                                QT_sb[pr][64 * x:64 * x + 64, qsl],
                                start=True, stop=True,
                                tile_position=(64 * x, 0))
                        if chunk_info[c] != "full":
                            mb = mbpool.tile([P, 512], F32, tag="mb", name=f"mb{c}")
                            nc.sync.dma_start(mb[:], maskbias[chunk_info[c]])
                            for x in range(2):
                                nc.vector.tensor_add(
                                    sT[:, 512 * x:512 * x + 512],
                                    sT[:, 512 * x:512 * x + 512], mb[:])
                        pt = ptpool.tile([P, 1024], BF16, tag="pt", name=f"pt{c}")
                        nc.scalar.activation(pt[:], sT[:], AF.Exp, scale=SCALE)
                        pts[c] = pt

                    def emit_av(c):
                        pt = pts.pop(c)
                        for x, h in enumerate((ha, hb)):
                            nc.tensor.matmul(
                                av[x][0:DK + 1, :],
                                V_sb[h][:, c, :],
                                pt[:, 512 * x:512 * x + 512],
                                start=(c == 0), stop=(c == n_chunks - 1))

                    # software pipeline: S^T runs 2 chunks ahead of AV
                    for c in range(n_chunks):
                        emit_st(c)
                        if c >= 2:
                            emit_av(c - 2)
                    emit_av(n_chunks - 2)
                    emit_av(n_chunks - 1)

                    for x, h in enumerate((ha, hb)):
                        avs = avstage.tile([DK + 1, 512], F32, tag="avs",
                                           name=f"avs{h}")
                        nc.vector.tensor_copy(avs[:], av[x][0:DK + 1, :])
                        avss[h] = avs

                # reciprocal of the 4 rowsum rows: gather via DRAM to use all
                # DVE lanes, recip once, scatter back for the broadcast DMAs
                rsd = dramp.tile([HC, 512], F32, tag="rsd")
                for h in range(HC):
                    nc.sync.dma_start(rsd[h:h + 1, :], avss[h][DK:DK + 1, :])
                rsg = bcast.tile([P, 16], F32, tag="rsg")
                gather_ap = bass.AP(
                    tensor=rsd.tensor, offset=rsd[:].offset,
                    ap=[[1, P], [512, HC], [P, 4]])
                nc.sync.dma_start(out=rsg[:], in_=gather_ap)
                nc.vector.reciprocal(rsg[:], rsg[:])
                rrd = dramp.tile([HC, 512], F32, tag="rrd")
                scatter_ap = bass.AP(
                    tensor=rrd.tensor, offset=rrd[:].offset,
                    ap=[[1, P], [512, HC], [P, 4]])
                nc.sync.dma_start(out=scatter_ap, in_=rsg[:])
                for h in range(HC):
                    bc = bcast.tile([DK, 512], F32, tag="bc", name=f"bc{h}")
                    bcast_ap = bass.AP(
                        tensor=rrd.tensor, offset=rrd[:].offset + h * 512,
                        ap=[[0, DK], [1, 512]])
                    nc.gpsimd.dma_start(out=bc[:], in_=bcast_ap)
                    at = attnT_pool.tile([DK, 512], BF16, tag="at", name=f"at{h}")
                    nc.gpsimd.tensor_tensor(at[:], avss[h][0:DK, :], bc[:],
                                            mybir.AluOpType.mult)
                    att[h] = at

                if not do_oproj:
                    zt = ostage.tile([P, 512], F32, tag="ost")
                    nc.vector.memset(zt[:], 0.0)
                    nc.sync.dma_start(outp[0:P, 0:512], zt[:])
                    for h in range(HC):
                        sink = ostage.tile([DK, 512], BF16, tag="atsink",
                                           name=f"sink{h}")
                        nc.vector.tensor_copy(sink[:], att[h][:])
                    return
                for ns in range(2):
                    for sub in range(4):
                        po = gen_ps.tile([P, 512], F32, tag="gen",
                                         name=f"po{ns}{sub}")
                        for h in range(HC):
                            nc.tensor.matmul(
                                po[:],
                                att[h][:, sub * P:(sub + 1) * P],
                                wo_sb[:, h, ns * 512:(ns + 1) * 512],
                                start=(h == 0), stop=(h == HC - 1))
                        ost = ostage.tile([P, 512], F32, tag="ost",
                                          name=f"ost{ns}{sub}")
                        nc.vector.tensor_copy(ost[:], po[:])
                        nc.sync.dma_start(
                            outp[qs + sub * P:qs + (sub + 1) * P,
                                 ns * 512:(ns + 1) * 512], ost[:])

            _nt = NT if _bisect in ("full", "attn") else (
                0 if _bisect == "proj" else 1)
            _do_oproj = _bisect == "full"
            for ti in range(NT):
                emit_proj(ti)
                if ti == NT - 1:
                    for p in range(2):
                        nc.sync.dma_start(kTnew[p], kTn_sb[p][:])
                    for h in range(HC):
                        nc.sync.dma_start(
                            vnew[h].rearrange("(c p) d -> p c d", p=P),
                            V_sb[h][:, TP // P:NKC, 0:DK])
                if ti < _nt:
                    emit_attn(ti, _do_oproj)
            if _bisect == "proj":
                zt0 = ostage.tile([P, 512], F32, tag="ost")
                nc.vector.memset(zt0[:], 0.0)
                nc.sync.dma_start(outp[0:P, 0:512], zt0[:])

    nc.finalize()
    return nc


def _get_kernel(plan_key, plan, n_mb):
    import os as _os
    plan_key = (plan_key, _os.environ.get("KBISECT", "full"))
    if plan_key not in _BUILD_CACHE:
        _BUILD_CACHE[plan_key] = _build(plan_key, plan, n_mb)
    return _BUILD_CACHE[plan_key]


def kernel(query, key, value, past_K, past_V, mask, Wq, bq, Wk, bk, Wv, bv,
           Wo, bo, _trace=False):
    global LAST_RESULT
    from concourse import bass_utils

    query = np.asarray(query, dtype=np.float32)
    key = np.asarray(key, dtype=np.float32)
    value = np.asarray(value, dtype=np.float32)
    past_K = np.asarray(past_K, dtype=np.float32)
    past_V = np.asarray(past_V, dtype=np.float32)
    Wq = np.asarray(Wq, dtype=np.float32)
    Wk = np.asarray(Wk, dtype=np.float32)
    Wv = np.asarray(Wv, dtype=np.float32)
    Wo = np.asarray(Wo, dtype=np.float32)
    bq = np.asarray(bq, dtype=np.float32)
    bk = np.asarray(bk, dtype=np.float32)
    bv = np.asarray(bv, dtype=np.float32)
    bo = np.asarray(bo, dtype=np.float32)

    ext, plan, mb = _analyze_mask(mask)
    plan_key = tuple(ext.tolist())
    nc = _get_kernel(plan_key, plan, mb.shape[0])

    # ---- host-side sharding / prep ----
    bf = ml_dtypes.bfloat16
    qT = [np.ascontiguousarray(query[b].T).astype(bf) for b in range(B)]
    kT = [np.ascontiguousarray(key[b].T).astype(bf) for b in range(B)]
    vT = [np.ascontiguousarray(value[b].T).astype(bf) for b in range(B)]

    in_maps = []
    for c in range(NCORES):
        b = c // (NCORES // B)
        g = c % (NCORES // B)
        hs = g * HC                      # first head of this core
        ds0 = hs * DK                    # first d_model dim of this core
        wq_l = Wq[ds0:ds0 + DLOC]        # [256, 1024]
        wk_l = Wk[ds0:ds0 + DLOC]
        wv_l = Wv[ds0:ds0 + DLOC]
        wo_l = Wo[:, ds0:ds0 + DLOC]     # [1024, 256]
        pkt = np.ascontiguousarray(
            past_K[b, hs:hs + HC].transpose(0, 2, 1)   # [4, 64, 2048]
        ).reshape(2, P, TP).astype(ml_dtypes.bfloat16)
        in_maps.append({
            "qT": qT[b],
            "kT": kT[b],
            "vT": vT[b],
            "wqT": np.ascontiguousarray(wq_l.T).astype(bf),
            "wkT": np.ascontiguousarray(wk_l.T).astype(bf),
            "wvT": np.ascontiguousarray(wv_l.T).astype(bf),
            "woT": np.ascontiguousarray(wo_l.T).astype(bf),
            "bqc": np.ascontiguousarray(bq[ds0:ds0 + DLOC].reshape(2, P).T),
            "bkc": np.ascontiguousarray(bk[ds0:ds0 + DLOC].reshape(2, P).T),
            "bvr": np.ascontiguousarray(bv[ds0:ds0 + DLOC].reshape(1, DLOC)).astype(bf),
            "pastKT": pkt,
            "pastV": np.ascontiguousarray(past_V[b, hs:hs + HC]).astype(bf),
            "maskbias": mb,
        })

    res = bass_utils.run_bass_kernel_spmd(
        nc, in_maps, core_ids=list(range(NCORES)), trace=_trace)
    LAST_RESULT = res

    # ---- host-side gather ----
    out = np.zeros((B, TQ, DM), np.float32)
    K = np.empty((B, NH, TKV, DK), np.float32)
    V = np.empty((B, NH, TKV, DK), np.float32)
    K[:, :, :TP] = past_K
    V[:, :, :TP] = past_V
    for c in range(NCORES):
        b = c // (NCORES // B)
        g = c % (NCORES // B)
        hs = g * HC
        r = res.results[c]
        out[b] += r["outp"]
        ktn = r["kTnew"].reshape(HC, DK, TQ)
        for h in range(HC):
            K[b, hs + h, TP:] = ktn[h].T
        V[b, hs:hs + HC, TP:] = r["vnew"].astype(np.float32)
    out += bo
    return out, K, V


# revision 22
# speedup vs baseline: 1.1985x; 1.1985x over previous
"""Multi-head attention with KV cache on 8 Trainium2 NeuronCores.

Sharding: 8 cores = 2 batches x 4 head-groups (4 heads each, tensor-parallel
over heads / d_model slices of the projections). Out-proj partials are summed
on the host; K/V cache shards per head with no cross-device comm.

Per-core kernel (Bass/Tile):
  - projections Q^T,K^T (transposed, bf16 out) and V (natural, fp32) via f32r
    matmuls; biases fused (per-partition add for Q/K, C=1 ones-matmul for V)
  - scores computed transposed S^T[kv, q] so softmax's P^T feeds AV directly:
    2 heads row-packed per PE pass (C=64 -> tile_position (0,0)/(64,0))
  - exp on ScalarE straight from PSUM (scale=1/8 fused), causal masking via
    additive -inf bias tiles on the diagonal chunks only; fully-masked kv
    chunks are skipped entirely
  - AV: lhsT=[V|1] f32r -> attn^T plus the softmax rowsum as a free 65th row
  - normalization: recip row via exp(-ln) on ScalarE, broadcast across
    partitions on GpSimd, one DVE multiply
  - out-proj consumes attn^T directly (C=64 per head, accumulated in PSUM)
"""

import sys

for _p in ("/opt/trn_rl_repo", "/opt/trn_rl_repo/concourse"):
    if _p not in sys.path:
        sys.path.insert(0, _p)

import numpy as np
import ml_dtypes

# ---- problem constants (hardcoded per contract) ----
B = 2
TQ = 2048
DM = 1024
NH = 16
DK = 64
TP = 2048
TKV = TP + TQ          # 4096
NCORES = 8
HC = NH // (NCORES // B)   # heads per core = 4
DLOC = HC * DK             # 256 local projection dims
P = 128
QT_TILE = 512              # q tile (free dim of S^T matmuls)
NT = TQ // QT_TILE         # 4 q tiles
NKC = TKV // P             # 32 kv chunks
SCALE = 1.0 / (DK ** 0.5)  # 0.125

_BUILD_CACHE = {}
LAST_RESULT = None         # BassKernelResults of the most recent run (for test.py)


def _analyze_mask(mask):
    """Per-row prefix extents + per-(tile, chunk) classification.

    Returns (ext, plan, mb) where plan[t] = (n_chunks, {chunk: ('full'|idx)})
    and mb is the stacked [n_partial, 128, QT_TILE] f32 additive-bias array
    in S^T layout (kv_local, q_local)."""
    m = np.asarray(mask).reshape(TQ, TKV).astype(bool)
    ext = m.sum(axis=1).astype(np.int64)
    # verify prefix form: row i is ones then zeros
    idx = np.arange(TKV)[None, :]
    assert (m == (idx < ext[:, None])).all(), "mask is not prefix-form per row"

    plan = []
    biases = []
    for t in range(NT):
        qs = t * QT_TILE
        e = ext[qs:qs + QT_TILE]
        emin, emax = int(e.min()), int(e.max())
        n_chunks = (emax + P - 1) // P
        chunks = {}
        for c in range(n_chunks):
            if (c + 1) * P <= emin:
                chunks[c] = "full"
            else:
                kv_idx = c * P + np.arange(P)[:, None]        # [128, 1]
                valid = kv_idx < e[None, :]                   # [128, 512]
                bias = np.where(valid, 0.0, -3.0e38).astype(np.float32)
                chunks[c] = len(biases)
                biases.append(bias)
        plan.append((n_chunks, chunks))
    if biases:
        mb = np.stack(biases)
    else:
        mb = np.zeros((1, P, QT_TILE), np.float32)
    return ext, plan, mb


def _build(plan_key, plan, n_mb):
    import concourse.bass as bass
    import concourse.mybir as mybir
    import concourse.tile as tile
    from concourse import bacc

    F32 = mybir.dt.float32
    F32R = mybir.dt.float32r
    BF16 = mybir.dt.bfloat16
    AF = mybir.ActivationFunctionType

    nc = bacc.Bacc(trn_type="TRN2")

    # ---- DRAM I/O ----
    qT = nc.dram_tensor("qT", [DM, TQ], BF16, kind="ExternalInput")
    kT = nc.dram_tensor("kT", [DM, TQ], BF16, kind="ExternalInput")
    vT = nc.dram_tensor("vT", [DM, TQ], BF16, kind="ExternalInput")
    wqT = nc.dram_tensor("wqT", [DM, DLOC], BF16, kind="ExternalInput")
    wkT = nc.dram_tensor("wkT", [DM, DLOC], BF16, kind="ExternalInput")
    wvT = nc.dram_tensor("wvT", [DM, DLOC], BF16, kind="ExternalInput")
    woT = nc.dram_tensor("woT", [DLOC, DM], BF16, kind="ExternalInput")
    bqc = nc.dram_tensor("bqc", [P, 2], F32, kind="ExternalInput")
    bkc = nc.dram_tensor("bkc", [P, 2], F32, kind="ExternalInput")
    bvr = nc.dram_tensor("bvr", [1, DLOC], BF16, kind="ExternalInput")
    pastKT = nc.dram_tensor("pastKT", [2, P, TP], BF16, kind="ExternalInput")
    pastV = nc.dram_tensor("pastV", [HC, TP, DK], BF16, kind="ExternalInput")
    maskbias = nc.dram_tensor("maskbias", [n_mb, P, QT_TILE], F32,
                              kind="ExternalInput")
    outp = nc.dram_tensor("outp", [TQ, DM], F32, kind="ExternalOutput")
    kTnew = nc.dram_tensor("kTnew", [2, P, TQ], F32, kind="ExternalOutput")
    vnew = nc.dram_tensor("vnew", [HC, TQ, DK], BF16, kind="ExternalOutput")

    with tile.TileContext(nc) as tc:
        with (
            tc.tile_pool(name="singles", bufs=1) as singles,
            tc.tile_pool(name="stage", bufs=2) as stage,
            tc.tile_pool(name="ptpool", bufs=3) as ptpool,
            tc.tile_pool(name="mbpool", bufs=2) as mbpool,
            tc.tile_pool(name="avstage", bufs=6) as avstage,
            tc.tile_pool(name="attnT", bufs=8) as attnT_pool,
            tc.tile_pool(name="bcast", bufs=2) as bcast,
            tc.tile_pool(name="ostage", bufs=3) as ostage,
            tc.tile_pool(name="dramp", bufs=4, space="DRAM") as dramp,
            tc.tile_pool(name="gen_ps", bufs=2, space="PSUM") as gen_ps,
            tc.tile_pool(name="st_ps", bufs=2, space="PSUM") as st_ps,
            tc.tile_pool(name="av_ps", bufs=2, space="PSUM") as av_ps,
        ):
            # ---- persistent SBUF tensors ----
            wq_sb = singles.tile([P, 8, DLOC], BF16)
            wk_sb = singles.tile([P, 8, DLOC], BF16)
            wv_sb = singles.tile([P, 8, DLOC], BF16)
            wo_sb = singles.tile([DK, HC, DM], BF16)
            nc.sync.dma_start(wq_sb[:], wqT.rearrange("(c p) m -> p c m", p=P))
            nc.sync.dma_start(wk_sb[:], wkT.rearrange("(c p) m -> p c m", p=P))
            nc.sync.dma_start(wv_sb[:], wvT.rearrange("(c p) m -> p c m", p=P))
            nc.sync.dma_start(wo_sb[:], woT.rearrange("(h p) m -> p h m", p=DK))
            bq_sb = singles.tile([P, 2], F32)
            bk_sb = singles.tile([P, 2], F32)
            bv_sb = singles.tile([1, DLOC], BF16)
            nc.sync.dma_start(bq_sb[:], bqc[:])
            nc.sync.dma_start(bk_sb[:], bkc[:])
            nc.sync.dma_start(bv_sb[:], bvr[:])
            ones_sb = singles.tile([P, P], BF16)
            nc.vector.memset(ones_sb[:], 1.0)

            KT_sb = [singles.tile([P, TKV], BF16, name=f"KT{i}") for i in range(2)]
            QT_sb = [singles.tile([P, TQ], BF16, name=f"QT{i}") for i in range(2)]
            V_sb = [singles.tile([P, NKC, DK + 1], BF16, name=f"V{i}")
                    for i in range(HC)]
            kTn_sb = [singles.tile([P, TQ], F32, name=f"kTn{i}") for i in range(2)]

            for p in range(2):
                nc.sync.dma_start(KT_sb[p][:, 0:TP], pastKT[p])
            for h in range(HC):
                nc.sync.dma_start(
                    V_sb[h][:, 0:TP // P, 0:DK],
                    pastV[h].rearrange("(c p) d -> p c d", p=P),
                )
                nc.vector.memset(V_sb[h][:, :, DK:DK + 1], 1.0)

            # ---- per-512-q-tile: projections then attention (interleaved) ----
            import os as _os
            _bisect = _os.environ.get("KBISECT", "full")

            def emit_proj(ti):
                ts0 = ti * QT_TILE
                tsl = slice(ts0, ts0 + QT_TILE)

                qst = stage.tile([P, 8, QT_TILE], BF16, tag="instage",
                                 name=f"qst{ti}")
                nc.sync.dma_start(
                    qst[:], qT[:, tsl].rearrange("(c p) t -> p c t", p=P))
                for m in range(2):
                    pq = gen_ps.tile([P, 512], F32, tag="gen", name=f"pq{ti}{m}")
                    for kc in range(8):
                        nc.tensor.matmul(
                            pq[:], wq_sb[:, kc, m * P:(m + 1) * P],
                            qst[:, kc, :],
                            start=(kc == 0), stop=(kc == 7))
                    nc.vector.tensor_scalar_add(
                        QT_sb[m][:, tsl], pq[:], bq_sb[:, m:m + 1])

                kst = stage.tile([P, 8, QT_TILE], BF16, tag="instage",
                                 name=f"kst{ti}")
                nc.sync.dma_start(
                    kst[:], kT[:, tsl].rearrange("(c p) t -> p c t", p=P))
                for m in range(2):
                    pk = gen_ps.tile([P, 512], F32, tag="gen", name=f"pk{ti}{m}")
                    for kc in range(8):
                        nc.tensor.matmul(
                            pk[:], wk_sb[:, kc, m * P:(m + 1) * P],
                            kst[:, kc, :],
                            start=(kc == 0), stop=(kc == 7))
                    nc.vector.tensor_scalar_add(
                        KT_sb[m][:, TP + ts0:TP + ts0 + QT_TILE], pk[:],
                        bk_sb[:, m:m + 1])
                    nc.vector.tensor_scalar_add(
                        kTn_sb[m][:, tsl], pk[:], bk_sb[:, m:m + 1])

                vst = stage.tile([P, 8, QT_TILE], BF16, tag="instage",
                                 name=f"vst{ti}")
                nc.sync.dma_start(
                    vst[:], vT[:, tsl].rearrange("(c p) t -> p c t", p=P))
                for sub in range(4):
                    pv = gen_ps.tile([P, 512], F32, tag="gen", name=f"pv{ti}{sub}")
                    for kc in range(8):
                        nc.tensor.matmul(
                            pv[:, 0:DLOC],
                            vst[:, kc, sub * P:(sub + 1) * P],
                            wv_sb[:, kc, :],
                            start=(kc == 0), stop=False)
                    nc.tensor.matmul(
                        pv[:, 0:DLOC], ones_sb[0:1, :],
                        bv_sb[:], start=False, stop=True)
                    ci = TP // P + ti * 4 + sub
                    for h in range(HC):
                        nc.vector.tensor_copy(
                            V_sb[h][:, ci, 0:DK], pv[:, h * DK:(h + 1) * DK])

            def emit_attn(ti, do_oproj):
                qs = ti * QT_TILE
                qsl = slice(qs, qs + QT_TILE)
                n_chunks, chunk_info = plan[ti]
                att = {}
                avss = {}
                for pr in range(2):
                    ha, hb = 2 * pr, 2 * pr + 1
                    av = [av_ps.tile([P, 512], F32, tag="av", name=f"av{i}")
                          for i in range(2)]
                    pts = {}

                    def emit_st(c):
                        sT = st_ps.tile([P, 1024], F32, tag="sT", name=f"sT{c}")
                        for x in range(2):
                            nc.tensor.matmul(
                                sT[:, 512 * x:512 * x + 512],
                                KT_sb[pr][64 * x:64 * x + 64, c * P:(c + 1) * P],
                                QT_sb[pr][64 * x:64 * x + 64, qsl],
                                start=True, stop=True,
                                tile_position=(64 * x, 0))
                        if chunk_info[c] != "full":
                            mb = mbpool.tile([P, 512], F32, tag="mb", name=f"mb{c}")
                            nc.sync.dma_start(mb[:], maskbias[chunk_info[c]])
                            for x in range(2):
                                nc.vector.tensor_add(
                                    sT[:, 512 * x:512 * x + 512],
                                    sT[:, 512 * x:512 * x + 512], mb[:])
                        pt = ptpool.tile([P, 1024], BF16, tag="pt", name=f"pt{c}")
                        nc.scalar.activation(pt[:], sT[:], AF.Exp, scale=SCALE)
                        pts[c] = pt

                    def emit_av(c):
                        pt = pts.pop(c)
                        for x, h in enumerate((ha, hb)):
                            nc.tensor.matmul(
                                av[x][0:DK + 1, :],
                                V_sb[h][:, c, :],
                                pt[:, 512 * x:512 * x + 512],
                                start=(c == 0), stop=(c == n_chunks - 1))

                    # software pipeline: S^T runs 2 chunks ahead of AV
                    for c in range(n_chunks):
                        emit_st(c)
                        if c >= 2:
                            emit_av(c - 2)
                    emit_av(n_chunks - 2)
                    emit_av(n_chunks - 1)

                    for x, h in enumerate((ha, hb)):
                        avs = avstage.tile([DK + 1, 512], F32, tag="avs",
                                           name=f"avs{h}")
                        nc.vector.tensor_copy(avs[:], av[x][0:DK + 1, :])
                        avss[h] = avs

                # reciprocal of the 4 rowsum rows: gather via DRAM to use all
                # DVE lanes, recip once, scatter back for the broadcast DMAs
                rsd = dramp.tile([HC, 512], F32, tag="rsd")
                for h in range(HC):
                    nc.gpsimd.dma_start(rsd[h:h + 1, :], avss[h][DK:DK + 1, :])
                rsg = bcast.tile([P, 16], F32, tag="rsg")
                gather_ap = bass.AP(
                    tensor=rsd.tensor, offset=rsd[:].offset,
                    ap=[[1, P], [512, HC], [P, 4]])
                nc.gpsimd.dma_start(out=rsg[:], in_=gather_ap)
                nc.vector.reciprocal(rsg[:], rsg[:])
                rrd = dramp.tile([HC, 512], F32, tag="rrd")
                scatter_ap = bass.AP(
                    tensor=rrd.tensor, offset=rrd[:].offset,
                    ap=[[1, P], [512, HC], [P, 4]])
                nc.gpsimd.dma_start(out=scatter_ap, in_=rsg[:])
                for h in range(HC):
                    bc = bcast.tile([DK, 512], F32, tag="bc", name=f"bc{h}")
                    bcast_ap = bass.AP(
                        tensor=rrd.tensor, offset=rrd[:].offset + h * 512,
                        ap=[[0, DK], [1, 512]])
                    nc.gpsimd.dma_start(out=bc[:], in_=bcast_ap)
                    at = attnT_pool.tile([DK, 512], BF16, tag="at", name=f"at{h}")
                    nc.gpsimd.tensor_tensor(at[:], avss[h][0:DK, :], bc[:],
                                            mybir.AluOpType.mult)
                    att[h] = at

                if not do_oproj:
                    zt = ostage.tile([P, 512], F32, tag="ost")
                    nc.vector.memset(zt[:], 0.0)
                    nc.sync.dma_start(outp[0:P, 0:512], zt[:])
                    for h in range(HC):
                        sink = ostage.tile([DK, 512], BF16, tag="atsink",
                                           name=f"sink{h}")
                        nc.vector.tensor_copy(sink[:], att[h][:])
                    return
                for ns in range(2):
                    for sub in range(4):
                        po = gen_ps.tile([P, 512], F32, tag="gen",
                                         name=f"po{ns}{sub}")
                        for h in range(HC):
                            nc.tensor.matmul(
                                po[:],
                                att[h][:, sub * P:(sub + 1) * P],
                                wo_sb[:, h, ns * 512:(ns + 1) * 512],
                                start=(h == 0), stop=(h == HC - 1))
                        ost = ostage.tile([P, 512], F32, tag="ost",
                                          name=f"ost{ns}{sub}")
                        nc.vector.tensor_copy(ost[:], po[:])
                        nc.sync.dma_start(
                            outp[qs + sub * P:qs + (sub + 1) * P,
                                 ns * 512:(ns + 1) * 512], ost[:])

            _nt = NT if _bisect in ("full", "attn") else (
                0 if _bisect == "proj" else 1)
            _do_oproj = _bisect == "full"
            emit_proj(0)
            for ti in range(NT):
                if ti + 1 < NT:
                    emit_proj(ti + 1)
                if ti < _nt:
                    emit_attn(ti, _do_oproj)
            for p in range(2):
                nc.scalar.dma_start(kTnew[p], kTn_sb[p][:])
            for h in range(HC):
                nc.scalar.dma_start(
                    vnew[h].rearrange("(c p) d -> p c d", p=P),
                    V_sb[h][:, TP // P:NKC, 0:DK])
            if _bisect == "proj":
                zt0 = ostage.tile([P, 512], F32, tag="ost")
                nc.vector.memset(zt0[:], 0.0)
                nc.sync.dma_start(outp[0:P, 0:512], zt0[:])

    nc.finalize()
    return nc


def _get_kernel(plan_key, plan, n_mb):
    import os as _os
    plan_key = (plan_key, _os.environ.get("KBISECT", "full"))
    if plan_key not in _BUILD_CACHE:
        _BUILD_CACHE[plan_key] = _build(plan_key, plan, n_mb)
    return _BUILD_CACHE[plan_key]


def kernel(query, key, value, past_K, past_V, mask, Wq, bq, Wk, bk, Wv, bv,
           Wo, bo, _trace=False):
    global LAST_RESULT
    from concourse import bass_utils

    query = np.asarray(query, dtype=np.float32)
    key = np.asarray(key, dtype=np.float32)
    value = np.asarray(value, dtype=np.float32)
    past_K = np.asarray(past_K, dtype=np.float32)
    past_V = np.asarray(past_V, dtype=np.float32)
    Wq = np.asarray(Wq, dtype=np.float32)
    Wk = np.asarray(Wk, dtype=np.float32)
    Wv = np.asarray(Wv, dtype=np.float32)
    Wo = np.asarray(Wo, dtype=np.float32)
    bq = np.asarray(bq, dtype=np.float32)
    bk = np.asarray(bk, dtype=np.float32)
    bv = np.asarray(bv, dtype=np.float32)
    bo = np.asarray(bo, dtype=np.float32)

    ext, plan, mb = _analyze_mask(mask)
    plan_key = tuple(ext.tolist())
    nc = _get_kernel(plan_key, plan, mb.shape[0])

    # ---- host-side sharding / prep ----
    bf = ml_dtypes.bfloat16
    qT = [np.ascontiguousarray(query[b].T).astype(bf) for b in range(B)]
    kT = [np.ascontiguousarray(key[b].T).astype(bf) for b in range(B)]
    vT = [np.ascontiguousarray(value[b].T).astype(bf) for b in range(B)]

    in_maps = []
    for c in range(NCORES):
        b = c // (NCORES // B)
        g = c % (NCORES // B)
        hs = g * HC                      # first head of this core
        ds0 = hs * DK                    # first d_model dim of this core
        wq_l = Wq[ds0:ds0 + DLOC]        # [256, 1024]
        wk_l = Wk[ds0:ds0 + DLOC]
        wv_l = Wv[ds0:ds0 + DLOC]
        wo_l = Wo[:, ds0:ds0 + DLOC]     # [1024, 256]
        pkt = np.ascontiguousarray(
            past_K[b, hs:hs + HC].transpose(0, 2, 1)   # [4, 64, 2048]
        ).reshape(2, P, TP).astype(ml_dtypes.bfloat16)
        in_maps.append({
            "qT": qT[b],
            "kT": kT[b],
            "vT": vT[b],
            "wqT": np.ascontiguousarray(wq_l.T).astype(bf),
            "wkT": np.ascontiguousarray(wk_l.T).astype(bf),
            "wvT": np.ascontiguousarray(wv_l.T).astype(bf),
            "woT": np.ascontiguousarray(wo_l.T).astype(bf),
            "bqc": np.ascontiguousarray(bq[ds0:ds0 + DLOC].reshape(2, P).T),
            "bkc": np.ascontiguousarray(bk[ds0:ds0 + DLOC].reshape(2, P).T),
            "bvr": np.ascontiguousarray(bv[ds0:ds0 + DLOC].reshape(1, DLOC)).astype(bf),
            "pastKT": pkt,
            "pastV": np.ascontiguousarray(past_V[b, hs:hs + HC]).astype(bf),
            "maskbias": mb,
        })

    res = bass_utils.run_bass_kernel_spmd(
        nc, in_maps, core_ids=list(range(NCORES)), trace=_trace)
    LAST_RESULT = res

    # ---- host-side gather ----
    out = np.zeros((B, TQ, DM), np.float32)
    K = np.empty((B, NH, TKV, DK), np.float32)
    V = np.empty((B, NH, TKV, DK), np.float32)
    K[:, :, :TP] = past_K
    V[:, :, :TP] = past_V
    for c in range(NCORES):
        b = c // (NCORES // B)
        g = c % (NCORES // B)
        hs = g * HC
        r = res.results[c]
        out[b] += r["outp"]
        ktn = r["kTnew"].reshape(HC, DK, TQ)
        for h in range(HC):
            K[b, hs + h, TP:] = ktn[h].T
        V[b, hs:hs + HC, TP:] = r["vnew"].astype(np.float32)
    out += bo
    return out, K, V
